# revision 23
# baseline (speedup 1.0000x reference)
"""Trainium2 Bass kernel for nn_MultiHeadCDGCN (v9, 174 us vs 292 us baseline).

Math (per batch b):
  t_w  = softmax(x, axis=T);  TAtt = sum_T(x * t_w)          [N, D]
  Q    = x @ W_Q.T                                           [T, N, D]
  K    = TAtt @ W_K.T ; V = TAtt @ W_V.T                     [N, D]
  S_th = Q_th @ K_h.T / sqrt(dh)   (per t, head h)           [N, N]
  out  = (relu(S) + I) @ V = relu(S) @ V + V                 [T, N, D]

Sharding: data-parallel over B across 8 NeuronCores (B == 8, one batch
per core); no collectives. The device computes out^T [D, T*N] per
batch; the host unshard step restores [T, N, D] layout.

Structure:
  - All matmuls and x itself in bf16 (fp32 matmuls lower to 2 HW
    passes; bf16 is 1 pass at ~1 col/cycle, and bf16 PE transposes are
    2x fp32). Total rel err ~3.3e-3 vs the 2e-2 gate.
  - GPSIMD cannot touch PSUM on TRN2, so every PSUM evacuation is on
    ACT/DVE; GPSIMD owns the SBUF-only sum_e accumulation; sum_xe
    accumulates on the PE as identity-matmul PSUM accumulation (one
    PSUM bank per accumulation group - sharing a bank corrupts it).
  - Phase C interleaves S and A@V instruction pairs with a 2-chunk
    software pipeline so the PE never idles (idle gaps trigger HAM
    re-throttle; HW then runs matmuls below full clock).
  - +V is fused into the po evacuation as scalar_tensor_tensor.
"""

import sys

import numpy as np

sys.path.insert(0, "/opt/trn_rl_repo")

import ml_dtypes  # noqa: E402

import concourse.bacc as bacc  # noqa: E402
import concourse.tile as tile  # noqa: E402
from concourse import mybir  # noqa: E402
from concourse.masks import make_identity  # noqa: E402
from concourse.bass_utils import run_bass_kernel_spmd  # noqa: E402

F32 = mybir.dt.float32
BF16 = mybir.dt.bfloat16
AF = mybir.ActivationFunctionType
ALU = mybir.AluOpType

B, T, N, D, H, DH = 8, 32, 256, 256, 8, 32
P = 128
NCHUNKS = 16  # tn chunks of 512 (2 frames each)
CHUNK_T = 2
CHUNK_TN = CHUNK_T * N  # 512

_CACHE: dict = {}


def _build_program():
    nc = bacc.Bacc()

    x_d = nc.dram_tensor("x", [T, N, D], BF16, kind="ExternalInput")
    wqt_d = nc.dram_tensor("wqt", [D, D], BF16, kind="ExternalInput")
    wkt_d = nc.dram_tensor("wkt", [D, D], BF16, kind="ExternalInput")
    wvt_d = nc.dram_tensor("wvt", [D, D], BF16, kind="ExternalInput")
    out_d = nc.dram_tensor("out", [D, T * N], F32, kind="ExternalOutput")

    with tile.TileContext(nc) as tc:
        with (
            tc.tile_pool(name="consts", bufs=1) as consts,
            tc.tile_pool(name="xa", bufs=3) as xa_pool,
            tc.tile_pool(name="xt", bufs=3) as xt_pool,
            tc.tile_pool(name="ew", bufs=8) as e_pool,
            tc.tile_pool(name="at", bufs=32) as a_pool,
            tc.tile_pool(name="ot", bufs=4) as o_pool,
            tc.tile_pool(name="misc", bufs=1) as misc,
        ):
            eye = consts.tile([P, P], F32)
            make_identity(nc, eye)
            eye_bf = consts.tile([P, P], BF16)
            nc.vector.tensor_copy(eye_bf, eye)

            # Weights [k, j], k split over 2 partition tiles, bf16.
            wqt_sb = consts.tile([P, 2, D], BF16)
            wkt_sb = consts.tile([P, 2, D], BF16)
            wvt_sb = consts.tile([P, 2, D], BF16)
            for w_sb, w_d in ((wqt_sb, wqt_d), (wkt_sb, wkt_d), (wvt_sb, wvt_d)):
                for kc in range(2):
                    nc.sync.dma_start(
                        out=w_sb[:, kc, :],
                        in_=w_d[kc * P : (kc + 1) * P, :],
                    )

            # Q.T strip [j, tn] resident, bf16 (j split over 2 tiles).
            qt_sb = consts.tile([P, 2, T * N], BF16)

            # sum_e wide accumulator (SBUF, GPSIMD-owned).
            acc_e = consts.tile([P, 2, CHUNK_TN], F32)
            nc.gpsimd.memset(acc_e, 0.0)

            # ============ Phase A + B (stat PSUM pool scoped) ============
            with (
                tc.tile_pool(name="ps_t", bufs=3, space="PSUM") as ps_t,
                tc.tile_pool(name="ps_q", bufs=3, space="PSUM") as ps_q,
                tc.tile_pool(name="ps_s", bufs=1, space="PSUM") as ps_s,
            ):
                # sum_xe accumulators: one full PSUM bank per dc so the two
                # long-lived accumulation groups never share a bank.
                acc_xe_t = [
                    ps_s.tile([P, CHUNK_TN], F32, name=f"accxe{dc}")
                    for dc in range(2)
                ]
                acc_xe = {dc: acc_xe_t[dc] for dc in range(2)}

                xe_strips = {}

                def stats_q_block(c, xt):
                    """xe stat-acc + Q projection for chunk c (PE work)."""
                    xe_t = xe_strips.pop(c)
                    for dc in range(2):
                        nc.tensor.matmul(
                            acc_xe[dc],
                            eye_bf,
                            xe_t[:, dc, :],
                            start=(c == 0),
                            stop=(c == NCHUNKS - 1),
                            skip_group_check=True,
                        )
                    for jc in range(2):
                        pq = ps_q.tile(
                            [P, CHUNK_TN], F32, tag="pq", name=f"pq{jc}"
                        )
                        for kc in range(2):
                            nc.tensor.matmul(
                                pq,
                                wqt_sb[:, kc, jc * P : (jc + 1) * P],
                                xt[:, kc, :],
                                start=(kc == 0),
                                stop=(kc == 1),
                            )
                        dst = qt_sb[:, jc, c * CHUNK_TN : (c + 1) * CHUNK_TN]
                        if c >= NCHUNKS - 2:
                            nc.scalar.activation(dst, pq, AF.Copy)
                        else:
                            nc.vector.tensor_copy(dst, pq)

                pipe = []  # [(c, xt), ...]
                for c in range(NCHUNKS):
                    t0 = c * CHUNK_T
                    xt = xt_pool.tile([P, 2, CHUNK_TN], BF16)
                    xrows = x_d[t0 : t0 + CHUNK_T].rearrange("t n d -> (t n) d")
                    xe_t = e_pool.tile([P, 2, CHUNK_TN], BF16, name="xe")
                    xe_strips[c] = xe_t
                    for dc in range(2):
                        # x^T chunk straight from DRAM via DMA transpose.
                        nc.sync.dma_start_transpose(
                            out=xt[:, dc, :],
                            in_=xrows[:, dc * P : (dc + 1) * P],
                        )
                        e_t = e_pool.tile([P, CHUNK_TN], BF16, name="e")
                        nc.scalar.activation(e_t, xt[:, dc, :], AF.Exp)
                        nc.vector.tensor_mul(xe_t[:, dc, :], xt[:, dc, :], e_t)
                        nc.gpsimd.tensor_add(
                            acc_e[:, dc, :], acc_e[:, dc, :], e_t
                        )
                    # Two-chunk software pipeline keeps the PE off the
                    # ACT/DVE critical path of recent chunks.
                    pipe.append((c, xt))
                    if len(pipe) > 2:
                        stats_q_block(*pipe.pop(0))
                for ent in pipe:
                    stats_q_block(*ent)

                # ---------------- Phase B: TAtt.T, K.T, V.T, V
                sum_e = misc.tile([P, 2, N], F32)
                for dc in range(2):
                    nc.vector.tensor_add(
                        sum_e[:, dc, :],
                        acc_e[:, dc, 0:N],
                        acc_e[:, dc, N : 2 * N],
                    )
                rec = misc.tile([P, 2, N], F32)
                sxe_sb = misc.tile([P, 2, CHUNK_TN], F32)
                sum_xe = misc.tile([P, 2, N], F32)
                tatt_t = misc.tile([P, 2, N], BF16)  # TAtt.T [d, n] bf16
                for dc in range(2):
                    nc.vector.tensor_copy(sxe_sb[:, dc, :], acc_xe[dc])
                    nc.vector.tensor_add(
                        sum_xe[:, dc, :],
                        sxe_sb[:, dc, 0:N],
                        sxe_sb[:, dc, N : 2 * N],
                    )
                    nc.vector.reciprocal(rec[:, dc, :], sum_e[:, dc, :])
                    nc.vector.scalar_tensor_tensor(
                        out=tatt_t[:, dc, :],
                        in0=sum_xe[:, dc, :],
                        scalar=1.0,
                        in1=rec[:, dc, :],
                        op0=ALU.mult,
                        op1=ALU.mult,
                    )

                kt_sb = consts.tile([P, 2, N], BF16)  # K.T [j, m] (pre-scaled)
                vt2 = consts.tile([P, 2, 2, N], F32)  # V.T doubled per hg
                for w_sb, is_v in ((wkt_sb, 0), (wvt_sb, 1)):
                    for jc in range(2):
                        pk = ps_q.tile([P, N], F32, tag="pq", name="pk")
                        for kc in range(2):
                            nc.tensor.matmul(
                                pk,
                                w_sb[:, kc, jc * P : (jc + 1) * P],
                                tatt_t[:, kc, :],
                                start=(kc == 0),
                                stop=(kc == 1),
                            )
                        if not is_v:
                            nc.vector.tensor_copy(kt_sb[:, jc, :], pk)
                        else:
                            for ti in range(2):
                                nc.vector.tensor_copy(vt2[:, jc, ti, :], pk)

                v_sb = consts.tile([P, 2, D], BF16)  # V [m, j]
                for mc in range(2):
                    pv = ps_q.tile([P, D], F32, tag="pq", name="pv")
                    for kc in range(2):
                        nc.tensor.matmul(
                            pv,
                            tatt_t[:, kc, mc * P : (mc + 1) * P],
                            wvt_sb[:, kc, :],
                            start=(kc == 0),
                            stop=(kc == 1),
                        )
                    nc.scalar.activation(v_sb[:, mc, :], pv, AF.Copy)

            # ============ Phase C: attention + output ============
            with (
                tc.tile_pool(name="ps_a", bufs=3, space="PSUM") as ps_a,
                tc.tile_pool(name="ps_o", bufs=2, space="PSUM") as ps_o,
            ):
                # 5 ACT : 3 DVE relu-evac split.
                relu_acts = (0, 2, 4, 6, 7)

                def s_pair(c, k, a_str):
                    """S matmuls + relu evac for head-pair k of chunk c."""
                    hg, mc, rp = k >> 2, (k >> 1) & 1, k & 1
                    ps2 = ps_a.tile(
                        [P, 2 * CHUNK_TN], F32, tag="psa", name=f"ps{k}"
                    )
                    for rh in range(2):
                        r = rp * 2 + rh
                        nc.tensor.matmul(
                            ps2[:, rh * CHUNK_TN : (rh + 1) * CHUNK_TN],
                            kt_sb[
                                r * 32 : (r + 1) * 32, hg, mc * P : (mc + 1) * P
                            ],
                            qt_sb[
                                r * 32 : (r + 1) * 32,
                                hg,
                                c * CHUNK_TN : (c + 1) * CHUNK_TN,
                            ],
                            start=True,
                            stop=True,
                            tile_position=(r * 32, 0),
                        )
                    a2 = a_pool.tile(
                        [P, 2 * CHUNK_TN], BF16, tag="at", name=f"a{k}"
                    )
                    if k in relu_acts:
                        nc.scalar.activation(a2, ps2, AF.Relu)
                    else:
                        nc.vector.tensor_scalar_max(a2, ps2, 0.0)
                    for rh in range(2):
                        a_str[(hg, rp * 2 + rh, mc)] = a2[
                            :, rh * CHUNK_TN : (rh + 1) * CHUNK_TN
                        ]

                def av_pair(c, k, a_str, pos):
                    """A@V matmul pair k (of 8) for chunk c."""
                    for i in range(2):
                        j = 2 * k + i
                        hg, mc, r = j >> 3, (j >> 2) & 1, j & 3
                        if r == 0 and mc == 0:
                            pos[hg] = ps_o.tile(
                                [P, CHUNK_TN], F32, tag="po", name=f"po{hg}"
                            )
                        h = hg * 4 + r
                        nc.tensor.matmul(
                            pos[hg][r * 32 : (r + 1) * 32, :],
                            v_sb[:, mc, h * 32 : (h + 1) * 32],
                            a_str[(hg, r, mc)],
                            start=(mc == 0),
                            stop=(mc == 1),
                            tile_position=(0, r * 32),
                            skip_group_check=True,
                        )

                def po_evac(c, pos):
                    """+V fused evacuation of po, then DMA of out^T strip."""
                    for hg in range(2):
                        o_sb = o_pool.tile([P, CHUNK_TN], F32, name=f"o{hg}")
                        nc.vector.scalar_tensor_tensor(
                            out=o_sb,
                            in0=pos[hg],
                            scalar=1.0,
                            in1=vt2[:, hg, :, :],
                            op0=ALU.mult,
                            op1=ALU.add,
                        )
                        nc.sync.dma_start(
                            out=out_d[
                                hg * P : (hg + 1) * P,
                                c * CHUNK_TN : (c + 1) * CHUNK_TN,
                            ],
                            in_=o_sb,
                        )

                astrs = {}
                poss = {}
                for c in range(NCHUNKS + 2):
                    if c < NCHUNKS:
                        astrs[c] = {}
                    if c - 2 >= 0:
                        poss[c - 2] = {}
                    for kk in range(4):
                        for k in (2 * kk, 2 * kk + 1):
                            if c < NCHUNKS:
                                s_pair(c, k, astrs[c])
                        for k in (2 * kk, 2 * kk + 1):
                            if c - 2 >= 0:
                                av_pair(c - 2, k, astrs[c - 2], poss[c - 2])
                    if c - 2 >= 0:
                        po_evac(c - 2, poss.pop(c - 2))
                        astrs.pop(c - 2)

    nc.finalize()
    return nc


def prepare_in_maps(inputs):
    x = np.ascontiguousarray(np.asarray(inputs["x"], dtype=np.float32))
    w_q = np.asarray(inputs["W_Q"], dtype=np.float32)
    w_k = np.asarray(inputs["W_K"], dtype=np.float32)
    w_v = np.asarray(inputs["W_V"], dtype=np.float32)

    wqt = np.ascontiguousarray(w_q.T).astype(ml_dtypes.bfloat16)
    wkt = np.ascontiguousarray(w_k.T * np.float32(1.0 / np.sqrt(DH))).astype(
        ml_dtypes.bfloat16
    )
    wvt = np.ascontiguousarray(w_v.T).astype(ml_dtypes.bfloat16)

    xb = x.astype(ml_dtypes.bfloat16)
    return [
        {"x": np.ascontiguousarray(xb[b]), "wqt": wqt, "wkt": wkt, "wvt": wvt}
        for b in range(B)
    ]


def kernel(**inputs) -> np.ndarray:
    if "nc" not in _CACHE:
        _CACHE["nc"] = _build_program()
    nc = _CACHE["nc"]

    in_maps = prepare_in_maps(inputs)
    res = run_bass_kernel_spmd(nc, in_maps, core_ids=list(range(B)))
    # Device emits out^T [D, T*N]; restore [T, N, D] during unshard.
    out = np.stack(
        [np.ascontiguousarray(res.results[b]["out"].T) for b in range(B)],
        axis=0,
    )
    return out.reshape(B, T, N, D)
